# revision 9
# baseline (speedup 1.0000x reference)
"""Fused ViT-style transformer block on 8 TRN2 NeuronCores.

Sharding: data-parallel over batch (32 batches -> 4 per core). Each core runs
the full block (LN1 -> QKV -> attention -> proj -> residual -> LN2 -> MLP ->
residual) on its 4 batches. No collectives.

Layout strategy per batch (N=577 tokens, D=768, H=12 heads, Dh=64):
  - x, x2, y: natural [token, feature] fp32, resident in one SBUF tile
    (residuals applied in place).
  - LayerNorm affine (g, b) folded into the following matmul weights on the
    host, so on-chip LN is just (x - mu) * rstd.
  - h (LN out) transposed via PE to hT [feature, token]; qT/kT computed
    feature-major directly (lhsT = W chunk), v computed token-major with an
    appended ones column so the PV matmul also yields softmax row sums.
  - Scores S^T [key j, query i] = kT.T @ qT; softmax skips max-subtraction
    (scores ~ N(0,1) after 1/8 scaling, safe in fp32 exp); exp on ACT engine
    with the 1/8 scale folded in, output bf16.
  - o = P @ v accumulated over key chunks; normalized per-token by the
    reciprocal of the appended row-sum column (per-partition scalar).
  - Matmul operands bf16 (PSUM accumulation fp32), activations/stats fp32.
"""

import numpy as np
import ml_dtypes
from contextlib import ExitStack

import concourse.bass as bass
import concourse.bacc as bacc
import concourse.tile as tile
from concourse import mybir
from concourse.masks import make_identity

F32 = mybir.dt.float32
BF16 = mybir.dt.bfloat16
AF = mybir.ActivationFunctionType
OP = mybir.AluOpType

B, N, D, H = 32, 577, 768, 12
DH = D // H            # 64
HID = 4 * D            # 3072
NCORES = 8
BPC = B // NCORES      # batches per core
P = 128
KD = D // P            # 6
KH = HID // P          # 24
NT = 5                 # token chunks per batch: 4x128 + 65
TSZ = [128, 128, 128, 128, 65]
TOF = [0, 128, 256, 384, 512]
EPS = 1e-5
SCALE = DH ** -0.5


def _bcast(ap_1d, p=P):
    """AP that reads a 1-D dram tensor broadcast across p partitions."""
    return bass.AP(
        tensor=ap_1d.tensor, offset=ap_1d.offset, ap=[[0, p]] + list(ap_1d.ap)
    )


def _layernorm(nc, stat, x_sb, out_sb, eps_sb):
    """out = (x - mean) * rsqrt(var + eps), per token chunk; out bf16."""
    for t in range(NT):
        ts_ = TSZ[t]
        st = stat.tile([P, 3, 6], F32, tag="bnst")
        mv = stat.tile([P, 2], F32, tag="bnmv")
        xin = x_sb[:ts_, t, :].rearrange("p (s q) -> p s q", s=3)
        for s in range(3):
            nc.vector.bn_stats(out=st[:ts_, s, :], in_=xin[:, s, :])
        nc.vector.bn_aggr(out=mv[:ts_], in_=st[:ts_])
        rstd = stat.tile([P, 1], F32, tag="rstd")
        nc.scalar.activation(
            out=rstd[:ts_], in_=mv[:ts_, 1:2], func=AF.Sqrt, bias=eps_sb[:ts_], scale=1.0
        )
        nc.vector.reciprocal(out=rstd[:ts_], in_=rstd[:ts_])
        nc.vector.tensor_scalar(
            out=out_sb[:ts_, t, :],
            in0=x_sb[:ts_, t, :],
            scalar1=mv[:ts_, 0:1],
            scalar2=rstd[:ts_],
            op0=OP.subtract,
            op1=OP.mult,
        )


def _transpose5x6(nc, ps_sm, ident, src_sb, dst_t6):
    """[token, 768] (5 chunks) -> [768, token] (6 chunks) via PE transposes."""
    for t in range(NT):
        ts_ = TSZ[t]
        for f in range(KD):
            tp = ps_sm.tile([P, P], BF16, tag="ps_sm")
            nc.tensor.transpose(
                out=tp[:, :ts_],
                in_=src_sb[:ts_, t, f * P : (f + 1) * P],
                identity=ident[:ts_, :ts_],
            )
            nc.vector.tensor_copy(
                out=dst_t6[:, f, TOF[t] : TOF[t] + ts_], in_=tp[:, :ts_]
            )


def _body(ctx, tc, d):
    nc = tc.nc

    const = ctx.enter_context(tc.tile_pool(name="const", bufs=1))
    xp = ctx.enter_context(tc.tile_pool(name="xp", bufs=2))
    hbf = ctx.enter_context(tc.tile_pool(name="hbf", bufs=2))
    t6 = ctx.enter_context(tc.tile_pool(name="t6", bufs=2))
    vvp = ctx.enter_context(tc.tile_pool(name="vv", bufs=1))
    qkp = ctx.enter_context(tc.tile_pool(name="qk", bufs=4))
    esp = ctx.enter_context(tc.tile_pool(name="es", bufs=1))
    h1p = ctx.enter_context(tc.tile_pool(name="h1", bufs=1))
    w1p = ctx.enter_context(tc.tile_pool(name="w1", bufs=4))
    stat = ctx.enter_context(tc.tile_pool(name="stat", bufs=4))
    ps_mm = ctx.enter_context(tc.tile_pool(name="ps_mm", bufs=2, space="PSUM"))
    ps_sm = ctx.enter_context(tc.tile_pool(name="ps_sm", bufs=4, space="PSUM"))

    # ---- one-time constants ----
    ident = const.tile([P, P], BF16)
    make_identity(nc, ident)
    eps_sb = const.tile([P, 1], F32)
    nc.vector.memset(eps_sb, EPS)

    wqkv_sb = const.tile([P, KD, 3 * D], BF16)
    nc.sync.dma_start(
        out=wqkv_sb, in_=d["w_qkv"][:].rearrange("(c p) f -> p c f", p=P)
    )
    wproj_sb = const.tile([P, KD, D], BF16)
    nc.sync.dma_start(
        out=wproj_sb, in_=d["w_proj"][:].rearrange("(c p) f -> p c f", p=P)
    )
    wfc2_sb = const.tile([P, KH, D], BF16)
    nc.sync.dma_start(
        out=wfc2_sb, in_=d["w_fc2"][:].rearrange("(c p) f -> p c f", p=P)
    )
    bqk_sb = const.tile([P, 12], F32)
    nc.sync.dma_start(out=bqk_sb, in_=d["b_qk"][:].rearrange("(c p) -> p c", p=P))
    bfc1_sb = const.tile([P, KH], F32)
    nc.sync.dma_start(out=bfc1_sb, in_=d["b_fc1"][:].rearrange("(c p) -> p c", p=P))
    bv_sb = const.tile([P, D], BF16)
    nc.sync.dma_start(out=bv_sb, in_=_bcast(d["b_v"][:]))
    bproj_sb = const.tile([P, D], BF16)
    nc.sync.dma_start(out=bproj_sb, in_=_bcast(d["b_proj"][:]))
    bfc2_sb = const.tile([P, D], BF16)
    nc.sync.dma_start(out=bfc2_sb, in_=_bcast(d["b_fc2"][:]))

    x_d, y_d = d["x"], d["y"]

    for b in range(BPC):
        # ---- load x ----
        x_sb = xp.tile([P, NT, D], F32, tag="x")
        nc.sync.dma_start(
            out=x_sb[:, 0:4, :],
            in_=x_d[b, 0:512, :].rearrange("(c p) f -> p c f", p=P),
        )
        nc.sync.dma_start(out=x_sb[0:65, 4, :], in_=x_d[b, 512:577, :])

        # ---- LN1 -> h ----
        h_sb = hbf.tile([P, NT, D], BF16, tag="hbf")
        _layernorm(nc, stat, x_sb, h_sb, eps_sb)

        # ---- transpose h -> hT ----
        hT = t6.tile([P, KD, N], BF16, tag="t6")
        _transpose5x6(nc, ps_sm, ident, h_sb, hT)

        # ---- v = h @ Wv + bv (token-major), append ones column ----
        v_sb = vvp.tile([P, NT, H, DH + 1], BF16, tag="vv")
        for t in range(NT):
            ts_ = TSZ[t]
            pv = ps_mm.tile([P, D], F32, tag="ps_mm")
            for k in range(KD):
                for n0, n1 in ((0, 512), (512, 768)):
                    nc.tensor.matmul(
                        pv[:ts_, n0:n1],
                        lhsT=hT[:, k, TOF[t] : TOF[t] + ts_],
                        rhs=wqkv_sb[:, k, 2 * D + n0 : 2 * D + n1],
                        start=(k == 0),
                        stop=(k == KD - 1),
                    )
            for hh in range(H):
                nc.vector.tensor_add(
                    out=v_sb[:ts_, t, hh, 0:DH],
                    in0=pv[:ts_, hh * DH : (hh + 1) * DH],
                    in1=bv_sb[:ts_, hh * DH : (hh + 1) * DH],
                )
            nc.vector.memset(v_sb[:ts_, t, :, DH : DH + 1], 1.0)

        # ---- attention, two heads per 128-feature chunk ----
        o_sb = hbf.tile([P, NT, D], BF16, tag="hbf")
        for hp in range(KD):
            qT = qkp.tile([P, N], BF16, tag="qk")
            kT = qkp.tile([P, N], BF16, tag="qk")
            for dst, base, col in ((qT, hp * P, hp), (kT, D + hp * P, KD + hp)):
                pq = ps_mm.tile([P, N], F32, tag="ps_mm")
                for k in range(KD):
                    for n0, n1 in ((0, 512), (512, N)):
                        nc.tensor.matmul(
                            pq[:, n0:n1],
                            lhsT=wqkv_sb[:, k, base : base + P],
                            rhs=hT[:, k, n0:n1],
                            start=(k == 0),
                            stop=(k == KD - 1),
                        )
                nc.vector.tensor_scalar_add(
                    out=dst, in0=pq, scalar1=bqk_sb[:, col : col + 1]
                )
            for sub in range(2):
                hh = hp * 2 + sub
                head_q = qT[sub * DH : (sub + 1) * DH, :]
                head_k = kT[sub * DH : (sub + 1) * DH, :]
                es = esp.tile([P, NT, N], BF16, tag="es")
                for j in range(NT):
                    js = TSZ[j]
                    ps_s = ps_mm.tile([P, N], F32, tag="ps_mm")
                    for n0, n1 in ((0, 512), (512, N)):
                        nc.tensor.matmul(
                            ps_s[:js, n0:n1],
                            lhsT=head_k[:, TOF[j] : TOF[j] + js],
                            rhs=head_q[:, n0:n1],
                            start=True,
                            stop=True,
                        )
                    nc.scalar.activation(
                        out=es[:js, j, :], in_=ps_s[:js, :], func=AF.Exp, scale=SCALE
                    )
                for t in range(NT):
                    ts_ = TSZ[t]
                    po = ps_sm.tile([P, DH + 1], F32, tag="ps_sm")
                    for j in range(NT):
                        js = TSZ[j]
                        nc.tensor.matmul(
                            po[:ts_, :],
                            lhsT=es[:js, j, TOF[t] : TOF[t] + ts_],
                            rhs=v_sb[:js, j, hh, :],
                            start=(j == 0),
                            stop=(j == NT - 1),
                        )
                    rcp = stat.tile([P, 1], F32, tag="rcp")
                    nc.vector.reciprocal(out=rcp[:ts_], in_=po[:ts_, DH : DH + 1])
                    nc.vector.tensor_scalar_mul(
                        out=o_sb[:ts_, t, hh * DH : (hh + 1) * DH],
                        in0=po[:ts_, 0:DH],
                        scalar1=rcp[:ts_],
                    )

        # ---- transpose o -> oT ----
        oT = t6.tile([P, KD, N], BF16, tag="t6")
        _transpose5x6(nc, ps_sm, ident, o_sb, oT)

        # ---- proj + residual into x_sb (x2 = x + o @ Wp + bp) ----
        for t in range(NT):
            ts_ = TSZ[t]
            pp = ps_mm.tile([P, D], F32, tag="ps_mm")
            for k in range(KD):
                for n0, n1 in ((0, 512), (512, 768)):
                    nc.tensor.matmul(
                        pp[:ts_, n0:n1],
                        lhsT=oT[:, k, TOF[t] : TOF[t] + ts_],
                        rhs=wproj_sb[:, k, n0:n1],
                        start=(k == 0),
                        stop=(k == KD - 1),
                    )
            nc.vector.tensor_add(
                out=x_sb[:ts_, t, :], in0=x_sb[:ts_, t, :], in1=pp[:ts_, :]
            )
            nc.vector.tensor_add(
                out=x_sb[:ts_, t, :], in0=x_sb[:ts_, t, :], in1=bproj_sb[:ts_, :]
            )

        # ---- LN2 -> h2, transpose ----
        h2_sb = hbf.tile([P, NT, D], BF16, tag="hbf")
        _layernorm(nc, stat, x_sb, h2_sb, eps_sb)
        h2T = t6.tile([P, KD, N], BF16, tag="t6")
        _transpose5x6(nc, ps_sm, ident, h2_sb, h2T)

        # ---- fc1 (feature-major out) + relu6 -> h1T; wfc1 streamed ----
        h1T = h1p.tile([P, KH, N], BF16, tag="h1")
        for f in range(KH):
            w1t = w1p.tile([P, KD, P], BF16, tag="w1")
            nc.sync.dma_start(
                out=w1t,
                in_=d["w_fc1"][:, f * P : (f + 1) * P].rearrange(
                    "(c p) f -> p c f", p=P
                ),
            )
            pf = ps_mm.tile([P, N], F32, tag="ps_mm")
            for k in range(KD):
                for n0, n1 in ((0, 512), (512, N)):
                    nc.tensor.matmul(
                        pf[:, n0:n1],
                        lhsT=w1t[:, k, :],
                        rhs=h2T[:, k, n0:n1],
                        start=(k == 0),
                        stop=(k == KD - 1),
                    )
            nc.scalar.activation(
                out=pf, in_=pf, func=AF.Relu, bias=bfc1_sb[:, f : f + 1], scale=1.0
            )
            nc.vector.tensor_scalar_min(out=h1T[:, f, :], in0=pf, scalar1=6.0)

        # ---- fc2 + residual -> y (into x_sb), DMA out ----
        for t in range(NT):
            ts_ = TSZ[t]
            pf2 = ps_mm.tile([P, D], F32, tag="ps_mm")
            for k in range(KH):
                for n0, n1 in ((0, 512), (512, 768)):
                    nc.tensor.matmul(
                        pf2[:ts_, n0:n1],
                        lhsT=h1T[:, k, TOF[t] : TOF[t] + ts_],
                        rhs=wfc2_sb[:, k, n0:n1],
                        start=(k == 0),
                        stop=(k == KH - 1),
                    )
            nc.vector.tensor_add(
                out=x_sb[:ts_, t, :], in0=x_sb[:ts_, t, :], in1=pf2[:ts_, :]
            )
            nc.vector.tensor_add(
                out=x_sb[:ts_, t, :], in0=x_sb[:ts_, t, :], in1=bfc2_sb[:ts_, :]
            )
        nc.sync.dma_start(
            out=y_d[b, 0:512, :].rearrange("(c p) f -> p c f", p=P),
            in_=x_sb[:, 0:4, :],
        )
        nc.sync.dma_start(out=y_d[b, 512:577, :], in_=x_sb[0:65, 4, :])


def build_nc():
    nc = bacc.Bacc("TRN2", target_bir_lowering=False, debug=False)
    d = {
        "x": nc.dram_tensor("x", [BPC, N, D], F32, kind="ExternalInput"),
        "w_qkv": nc.dram_tensor("w_qkv", [D, 3 * D], BF16, kind="ExternalInput"),
        "b_qk": nc.dram_tensor("b_qk", [2 * D], F32, kind="ExternalInput"),
        "b_v": nc.dram_tensor("b_v", [D], BF16, kind="ExternalInput"),
        "w_proj": nc.dram_tensor("w_proj", [D, D], BF16, kind="ExternalInput"),
        "b_proj": nc.dram_tensor("b_proj", [D], BF16, kind="ExternalInput"),
        "w_fc1": nc.dram_tensor("w_fc1", [D, HID], BF16, kind="ExternalInput"),
        "b_fc1": nc.dram_tensor("b_fc1", [HID], F32, kind="ExternalInput"),
        "w_fc2": nc.dram_tensor("w_fc2", [HID, D], BF16, kind="ExternalInput"),
        "b_fc2": nc.dram_tensor("b_fc2", [D], BF16, kind="ExternalInput"),
        "y": nc.dram_tensor("y", [BPC, N, D], F32, kind="ExternalOutput"),
    }
    with tile.TileContext(nc) as tc:
        with ExitStack() as ctx:
            _body(ctx, tc, d)
    nc.compile()
    return nc


def host_inputs(inputs):
    """Fold LN affine params into weights; cast matmul operands to bf16."""
    bf = ml_dtypes.bfloat16
    f32 = np.float32
    g1 = np.asarray(inputs["ln1_g"], f32)
    b1 = np.asarray(inputs["ln1_b"], f32)
    g2 = np.asarray(inputs["ln2_g"], f32)
    b2 = np.asarray(inputs["ln2_b"], f32)
    w_qkv = np.asarray(inputs["w_qkv"], f32)
    w_fc1 = np.asarray(inputs["w_fc1"], f32)
    b_qkv_eff = np.asarray(inputs["b_qkv"], f32) + b1 @ w_qkv
    b_fc1_eff = np.asarray(inputs["b_fc1"], f32) + b2 @ w_fc1
    return {
        "w_qkv": (g1[:, None] * w_qkv).astype(bf),
        "b_qk": b_qkv_eff[: 2 * D].astype(f32),
        "b_v": b_qkv_eff[2 * D :].astype(bf),
        "w_proj": np.asarray(inputs["w_proj"], f32).astype(bf),
        "b_proj": np.asarray(inputs["b_proj"], f32).astype(bf),
        "w_fc1": (g2[:, None] * w_fc1).astype(bf),
        "b_fc1": b_fc1_eff.astype(f32),
        "w_fc2": np.asarray(inputs["w_fc2"], f32).astype(bf),
        "b_fc2": np.asarray(inputs["b_fc2"], f32).astype(bf),
    }


_CACHE = {}


def get_runner():
    """Build (once) the bass module and a persistent 8-core PJRT runner."""
    if "runner" not in _CACHE:
        from concourse import bass2jax

        nc = build_nc()

        def run(in_maps):
            return bass2jax.run_bass_via_pjrt(nc, in_maps, n_cores=NCORES)

        _CACHE["runner"] = run
        _CACHE["nc"] = nc
    return _CACHE["runner"]


def make_in_maps(inputs):
    x = np.asarray(inputs["x"], np.float32)
    shared = host_inputs(inputs)
    return [
        {"x": np.ascontiguousarray(x[c * BPC : (c + 1) * BPC]), **shared}
        for c in range(NCORES)
    ]


def kernel(**inputs):
    run = get_runner()
    in_maps = make_in_maps(inputs)
    res = run(in_maps)
    y = np.concatenate([np.asarray(r["y"]) for r in res], axis=0)
    return y.astype(np.float32)


# revision 12
# speedup vs baseline: 57.9999x; 57.9999x over previous
"""Fused ViT-style transformer block on 8 TRN2 NeuronCores.

Sharding: data-parallel over batch (32 batches -> 4 per core). Each core runs
the full block (LN1 -> QKV -> attention -> proj -> residual -> LN2 -> MLP ->
residual) on its 4 batches. No collectives.

Layout strategy per batch (N=577 tokens, D=768, H=12 heads, Dh=64):
  - x, x2, y: natural [token, feature] fp32, resident in one SBUF tile
    (residuals applied in place).
  - LayerNorm affine (g, b) folded into the following matmul weights on the
    host, so on-chip LN is just (x - mu) * rstd.
  - h (LN out) transposed via PE to hT [feature, token]; qT/kT computed
    feature-major directly (lhsT = W chunk), v computed token-major with an
    appended ones column so the PV matmul also yields softmax row sums.
  - Scores S^T [key j, query i] = kT.T @ qT; softmax skips max-subtraction
    (scores ~ N(0,1) after 1/8 scaling, safe in fp32 exp); exp on ACT engine
    with the 1/8 scale folded in, output bf16.
  - o = P @ v accumulated over key chunks; normalized per-token by the
    reciprocal of the appended row-sum column (per-partition scalar).
  - Matmul operands bf16 (PSUM accumulation fp32), activations/stats fp32.
"""

import time

import numpy as np
import ml_dtypes
from contextlib import ExitStack

import concourse.bass as bass
import concourse.bacc as bacc
import concourse.tile as tile
from concourse import mybir
from concourse.masks import make_identity

F32 = mybir.dt.float32
BF16 = mybir.dt.bfloat16
AF = mybir.ActivationFunctionType
OP = mybir.AluOpType

B, N, D, H = 32, 577, 768, 12
DH = D // H            # 64
HID = 4 * D            # 3072
NCORES = 8
BPC = B // NCORES      # batches per core
P = 128
KD = D // P            # 6
KH = HID // P          # 24
NT = 5                 # token chunks per batch: 4x128 + 65
TSZ = [128, 128, 128, 128, 65]
TOF = [0, 128, 256, 384, 512]
EPS = 1e-5
SCALE = DH ** -0.5


def _bcast(ap_1d, p=P):
    """AP that reads a 1-D dram tensor broadcast across p partitions."""
    return bass.AP(
        tensor=ap_1d.tensor, offset=ap_1d.offset, ap=[[0, p]] + list(ap_1d.ap)
    )


def _layernorm(nc, stat, x_sb, out_sb, eps_sb):
    """out = (x - mean) * rsqrt(var + eps), per token chunk; out bf16."""
    for t in range(NT):
        ts_ = TSZ[t]
        st = stat.tile([P, 3, 6], F32, tag="bnst")
        mv = stat.tile([P, 2], F32, tag="bnmv")
        xin = x_sb[:ts_, t, :].rearrange("p (s q) -> p s q", s=3)
        for s in range(3):
            nc.vector.bn_stats(out=st[:ts_, s, :], in_=xin[:, s, :])
        nc.vector.bn_aggr(out=mv[:ts_], in_=st[:ts_])
        rstd = stat.tile([P, 1], F32, tag="rstd")
        nc.scalar.activation(
            out=rstd[:ts_], in_=mv[:ts_, 1:2], func=AF.Sqrt, bias=eps_sb[:ts_], scale=1.0
        )
        nc.vector.reciprocal(out=rstd[:ts_], in_=rstd[:ts_])
        nc.vector.tensor_scalar(
            out=out_sb[:ts_, t, :],
            in0=x_sb[:ts_, t, :],
            scalar1=mv[:ts_, 0:1],
            scalar2=rstd[:ts_],
            op0=OP.subtract,
            op1=OP.mult,
        )


def _transpose5x6(nc, ps_sm, ident, src_sb, dst_t6):
    """[token, 768] (5 chunks) -> [768, token] (6 chunks) via PE transposes."""
    for t in range(NT):
        ts_ = TSZ[t]
        for f in range(KD):
            tp = ps_sm.tile([P, P], BF16, tag="ps_sm")
            nc.tensor.transpose(
                out=tp[:, :ts_],
                in_=src_sb[:ts_, t, f * P : (f + 1) * P],
                identity=ident[:ts_, :ts_],
            )
            nc.vector.tensor_copy(
                out=dst_t6[:, f, TOF[t] : TOF[t] + ts_], in_=tp[:, :ts_]
            )


def _body(ctx, tc, d):
    nc = tc.nc

    const = ctx.enter_context(tc.tile_pool(name="const", bufs=1))
    xp = ctx.enter_context(tc.tile_pool(name="xp", bufs=2))
    hbf = ctx.enter_context(tc.tile_pool(name="hbf", bufs=2))
    t6 = ctx.enter_context(tc.tile_pool(name="t6", bufs=2))
    vvp = ctx.enter_context(tc.tile_pool(name="vv", bufs=1))
    qkp = ctx.enter_context(tc.tile_pool(name="qk", bufs=4))
    esp = ctx.enter_context(tc.tile_pool(name="es", bufs=1))
    h1p = ctx.enter_context(tc.tile_pool(name="h1", bufs=1))
    w1p = ctx.enter_context(tc.tile_pool(name="w1", bufs=4))
    stat = ctx.enter_context(tc.tile_pool(name="stat", bufs=4))
    ps_mm = ctx.enter_context(tc.tile_pool(name="ps_mm", bufs=2, space="PSUM"))
    ps_sm = ctx.enter_context(tc.tile_pool(name="ps_sm", bufs=4, space="PSUM"))

    # ---- one-time constants ----
    ident = const.tile([P, P], BF16)
    make_identity(nc, ident)
    eps_sb = const.tile([P, 1], F32)
    nc.vector.memset(eps_sb, EPS)

    wqkv_sb = const.tile([P, KD, 3 * D], BF16)
    nc.sync.dma_start(
        out=wqkv_sb, in_=d["w_qkv"][:].rearrange("(c p) f -> p c f", p=P)
    )
    wproj_sb = const.tile([P, KD, D], BF16)
    nc.sync.dma_start(
        out=wproj_sb, in_=d["w_proj"][:].rearrange("(c p) f -> p c f", p=P)
    )
    wfc2_sb = const.tile([P, KH, D], BF16)
    nc.sync.dma_start(
        out=wfc2_sb, in_=d["w_fc2"][:].rearrange("(c p) f -> p c f", p=P)
    )
    bqk_sb = const.tile([P, 12], F32)
    nc.sync.dma_start(out=bqk_sb, in_=d["b_qk"][:].rearrange("(c p) -> p c", p=P))
    bfc1_sb = const.tile([P, KH], F32)
    nc.sync.dma_start(out=bfc1_sb, in_=d["b_fc1"][:].rearrange("(c p) -> p c", p=P))
    bv_sb = const.tile([P, D], BF16)
    nc.sync.dma_start(out=bv_sb, in_=_bcast(d["b_v"][:]))
    bproj_sb = const.tile([P, D], BF16)
    nc.sync.dma_start(out=bproj_sb, in_=_bcast(d["b_proj"][:]))
    bfc2_sb = const.tile([P, D], BF16)
    nc.sync.dma_start(out=bfc2_sb, in_=_bcast(d["b_fc2"][:]))

    x_d, y_d = d["x"], d["y"]

    for b in range(BPC):
        # ---- load x ----
        x_sb = xp.tile([P, NT, D], F32, tag="x")
        nc.sync.dma_start(
            out=x_sb[:, 0:4, :],
            in_=x_d[b, 0:512, :].rearrange("(c p) f -> p c f", p=P),
        )
        nc.sync.dma_start(out=x_sb[0:65, 4, :], in_=x_d[b, 512:577, :])

        # ---- LN1 -> h ----
        h_sb = hbf.tile([P, NT, D], BF16, tag="hbf")
        _layernorm(nc, stat, x_sb, h_sb, eps_sb)

        # ---- transpose h -> hT ----
        hT = t6.tile([P, KD, N], BF16, tag="t6")
        _transpose5x6(nc, ps_sm, ident, h_sb, hT)

        # ---- v = h @ Wv + bv (token-major), append ones column ----
        v_sb = vvp.tile([P, NT, H, DH + 1], BF16, tag="vv")
        for t in range(NT):
            ts_ = TSZ[t]
            pv = ps_mm.tile([P, D], F32, tag="ps_mm")
            for k in range(KD):
                for n0, n1 in ((0, 512), (512, 768)):
                    nc.tensor.matmul(
                        pv[:ts_, n0:n1],
                        lhsT=hT[:, k, TOF[t] : TOF[t] + ts_],
                        rhs=wqkv_sb[:, k, 2 * D + n0 : 2 * D + n1],
                        start=(k == 0),
                        stop=(k == KD - 1),
                    )
            for hh in range(H):
                nc.vector.tensor_add(
                    out=v_sb[:ts_, t, hh, 0:DH],
                    in0=pv[:ts_, hh * DH : (hh + 1) * DH],
                    in1=bv_sb[:ts_, hh * DH : (hh + 1) * DH],
                )
            nc.vector.memset(v_sb[:ts_, t, :, DH : DH + 1], 1.0)

        # ---- attention, two heads per 128-feature chunk ----
        o_sb = hbf.tile([P, NT, D], BF16, tag="hbf")
        for hp in range(KD):
            qT = qkp.tile([P, N], BF16, tag="qk")
            kT = qkp.tile([P, N], BF16, tag="qk")
            for dst, base, col in ((qT, hp * P, hp), (kT, D + hp * P, KD + hp)):
                pq = ps_mm.tile([P, N], F32, tag="ps_mm")
                for k in range(KD):
                    for n0, n1 in ((0, 512), (512, N)):
                        nc.tensor.matmul(
                            pq[:, n0:n1],
                            lhsT=wqkv_sb[:, k, base : base + P],
                            rhs=hT[:, k, n0:n1],
                            start=(k == 0),
                            stop=(k == KD - 1),
                        )
                nc.vector.tensor_scalar_add(
                    out=dst, in0=pq, scalar1=bqk_sb[:, col : col + 1]
                )
            for sub in range(2):
                hh = hp * 2 + sub
                head_q = qT[sub * DH : (sub + 1) * DH, :]
                head_k = kT[sub * DH : (sub + 1) * DH, :]
                es = esp.tile([P, NT, N], BF16, tag="es")
                for j in range(NT):
                    js = TSZ[j]
                    ps_s = ps_mm.tile([P, N], F32, tag="ps_mm")
                    for n0, n1 in ((0, 512), (512, N)):
                        nc.tensor.matmul(
                            ps_s[:js, n0:n1],
                            lhsT=head_k[:, TOF[j] : TOF[j] + js],
                            rhs=head_q[:, n0:n1],
                            start=True,
                            stop=True,
                        )
                    nc.scalar.activation(
                        out=es[:js, j, :], in_=ps_s[:js, :], func=AF.Exp, scale=SCALE
                    )
                for t in range(NT):
                    ts_ = TSZ[t]
                    po = ps_sm.tile([P, DH + 1], F32, tag="ps_sm")
                    for j in range(NT):
                        js = TSZ[j]
                        nc.tensor.matmul(
                            po[:ts_, :],
                            lhsT=es[:js, j, TOF[t] : TOF[t] + ts_],
                            rhs=v_sb[:js, j, hh, :],
                            start=(j == 0),
                            stop=(j == NT - 1),
                        )
                    rcp = stat.tile([P, 1], F32, tag="rcp")
                    nc.vector.reciprocal(out=rcp[:ts_], in_=po[:ts_, DH : DH + 1])
                    nc.vector.tensor_scalar_mul(
                        out=o_sb[:ts_, t, hh * DH : (hh + 1) * DH],
                        in0=po[:ts_, 0:DH],
                        scalar1=rcp[:ts_],
                    )

        # ---- transpose o -> oT ----
        oT = t6.tile([P, KD, N], BF16, tag="t6")
        _transpose5x6(nc, ps_sm, ident, o_sb, oT)

        # ---- proj + residual into x_sb (x2 = x + o @ Wp + bp) ----
        for t in range(NT):
            ts_ = TSZ[t]
            pp = ps_mm.tile([P, D], F32, tag="ps_mm")
            for k in range(KD):
                for n0, n1 in ((0, 512), (512, 768)):
                    nc.tensor.matmul(
                        pp[:ts_, n0:n1],
                        lhsT=oT[:, k, TOF[t] : TOF[t] + ts_],
                        rhs=wproj_sb[:, k, n0:n1],
                        start=(k == 0),
                        stop=(k == KD - 1),
                    )
            nc.vector.tensor_add(
                out=x_sb[:ts_, t, :], in0=x_sb[:ts_, t, :], in1=pp[:ts_, :]
            )
            nc.vector.tensor_add(
                out=x_sb[:ts_, t, :], in0=x_sb[:ts_, t, :], in1=bproj_sb[:ts_, :]
            )

        # ---- LN2 -> h2, transpose ----
        h2_sb = hbf.tile([P, NT, D], BF16, tag="hbf")
        _layernorm(nc, stat, x_sb, h2_sb, eps_sb)
        h2T = t6.tile([P, KD, N], BF16, tag="t6")
        _transpose5x6(nc, ps_sm, ident, h2_sb, h2T)

        # ---- fc1 (feature-major out) + relu6 -> h1T; wfc1 streamed ----
        h1T = h1p.tile([P, KH, N], BF16, tag="h1")
        for f in range(KH):
            w1t = w1p.tile([P, KD, P], BF16, tag="w1")
            nc.sync.dma_start(
                out=w1t,
                in_=d["w_fc1"][:, f * P : (f + 1) * P].rearrange(
                    "(c p) f -> p c f", p=P
                ),
            )
            pf = ps_mm.tile([P, N], F32, tag="ps_mm")
            for k in range(KD):
                for n0, n1 in ((0, 512), (512, N)):
                    nc.tensor.matmul(
                        pf[:, n0:n1],
                        lhsT=w1t[:, k, :],
                        rhs=h2T[:, k, n0:n1],
                        start=(k == 0),
                        stop=(k == KD - 1),
                    )
            nc.scalar.activation(
                out=pf, in_=pf, func=AF.Relu, bias=bfc1_sb[:, f : f + 1], scale=1.0
            )
            nc.vector.tensor_scalar_min(out=h1T[:, f, :], in0=pf, scalar1=6.0)

        # ---- fc2 + residual -> y (into x_sb), DMA out ----
        for t in range(NT):
            ts_ = TSZ[t]
            pf2 = ps_mm.tile([P, D], F32, tag="ps_mm")
            for k in range(KH):
                for n0, n1 in ((0, 512), (512, 768)):
                    nc.tensor.matmul(
                        pf2[:ts_, n0:n1],
                        lhsT=h1T[:, k, TOF[t] : TOF[t] + ts_],
                        rhs=wfc2_sb[:, k, n0:n1],
                        start=(k == 0),
                        stop=(k == KH - 1),
                    )
            nc.vector.tensor_add(
                out=x_sb[:ts_, t, :], in0=x_sb[:ts_, t, :], in1=pf2[:ts_, :]
            )
            nc.vector.tensor_add(
                out=x_sb[:ts_, t, :], in0=x_sb[:ts_, t, :], in1=bfc2_sb[:ts_, :]
            )
        nc.sync.dma_start(
            out=y_d[b, 0:512, :].rearrange("(c p) f -> p c f", p=P),
            in_=x_sb[:, 0:4, :],
        )
        nc.sync.dma_start(out=y_d[b, 512:577, :], in_=x_sb[0:65, 4, :])


def build_nc():
    nc = bacc.Bacc("TRN2", target_bir_lowering=False, debug=False)
    d = {
        "x": nc.dram_tensor("x", [BPC, N, D], F32, kind="ExternalInput"),
        "w_qkv": nc.dram_tensor("w_qkv", [D, 3 * D], BF16, kind="ExternalInput"),
        "b_qk": nc.dram_tensor("b_qk", [2 * D], F32, kind="ExternalInput"),
        "b_v": nc.dram_tensor("b_v", [D], BF16, kind="ExternalInput"),
        "w_proj": nc.dram_tensor("w_proj", [D, D], BF16, kind="ExternalInput"),
        "b_proj": nc.dram_tensor("b_proj", [D], BF16, kind="ExternalInput"),
        "w_fc1": nc.dram_tensor("w_fc1", [D, HID], BF16, kind="ExternalInput"),
        "b_fc1": nc.dram_tensor("b_fc1", [HID], F32, kind="ExternalInput"),
        "w_fc2": nc.dram_tensor("w_fc2", [HID, D], BF16, kind="ExternalInput"),
        "b_fc2": nc.dram_tensor("b_fc2", [D], BF16, kind="ExternalInput"),
        "y": nc.dram_tensor("y", [BPC, N, D], F32, kind="ExternalOutput"),
    }
    with tile.TileContext(nc) as tc:
        with ExitStack() as ctx:
            _body(ctx, tc, d)
    nc.compile()
    return nc


def host_inputs(inputs):
    """Fold LN affine params into weights; cast matmul operands to bf16."""
    bf = ml_dtypes.bfloat16
    f32 = np.float32
    g1 = np.asarray(inputs["ln1_g"], f32)
    b1 = np.asarray(inputs["ln1_b"], f32)
    g2 = np.asarray(inputs["ln2_g"], f32)
    b2 = np.asarray(inputs["ln2_b"], f32)
    w_qkv = np.asarray(inputs["w_qkv"], f32)
    w_fc1 = np.asarray(inputs["w_fc1"], f32)
    b_qkv_eff = np.asarray(inputs["b_qkv"], f32) + b1 @ w_qkv
    b_fc1_eff = np.asarray(inputs["b_fc1"], f32) + b2 @ w_fc1
    return {
        "w_qkv": (g1[:, None] * w_qkv).astype(bf),
        "b_qk": b_qkv_eff[: 2 * D].astype(f32),
        "b_v": b_qkv_eff[2 * D :].astype(bf),
        "w_proj": np.asarray(inputs["w_proj"], f32).astype(bf),
        "b_proj": np.asarray(inputs["b_proj"], f32).astype(bf),
        "w_fc1": (g2[:, None] * w_fc1).astype(bf),
        "b_fc1": b_fc1_eff.astype(f32),
        "w_fc2": np.asarray(inputs["w_fc2"], f32).astype(bf),
        "b_fc2": np.asarray(inputs["b_fc2"], f32).astype(bf),
    }


_CACHE = {}


def get_runner():
    """Build (once) the bass module and a persistent 8-core PJRT runner.

    Mirrors bass2jax.run_bass_via_pjrt's multi-core branch, but caches the
    jitted shard_map callable so repeat calls don't re-trace/re-compile.
    """
    if "runner" in _CACHE:
        return _CACHE["runner"]

    import jax
    from jax.sharding import Mesh, PartitionSpec
    from jax.experimental.shard_map import shard_map
    from concourse import bass2jax, mybir as mb

    bass2jax.install_neuronx_cc_hook()
    nc = build_nc()

    partition_name = nc.partition_id_tensor.name if nc.partition_id_tensor else None
    in_names, out_names, out_avals = [], [], []
    for alloc in nc.m.functions[0].allocations:
        if not isinstance(alloc, mb.MemoryLocationSet):
            continue
        name = alloc.memorylocations[0].name
        if alloc.kind == "ExternalInput":
            if name != partition_name:
                in_names.append(name)
        elif alloc.kind == "ExternalOutput":
            out_names.append(name)
            out_avals.append(
                jax.core.ShapedArray(tuple(alloc.tensor_shape), mb.dt.np(alloc.dtype))
            )
    n_params = len(in_names)
    n_outs = len(out_names)
    all_names = in_names + out_names
    if partition_name is not None:
        all_names = all_names + [partition_name]

    def _body(*args):
        operands = list(args)
        if partition_name is not None:
            operands.append(bass2jax.partition_id_tensor())
        return tuple(
            bass2jax._bass_exec_p.bind(
                *operands,
                out_avals=tuple(out_avals),
                in_names=tuple(all_names),
                out_names=tuple(out_names),
                lowering_input_output_aliases=(),
                sim_require_finite=True,
                sim_require_nnan=True,
                nc=nc,
            )
        )

    devices = jax.devices()[:NCORES]
    mesh = Mesh(np.asarray(devices), ("core",))
    donate = tuple(range(n_params, n_params + n_outs))
    sharded = jax.jit(
        shard_map(
            _body,
            mesh=mesh,
            in_specs=(PartitionSpec("core"),) * (n_params + n_outs),
            out_specs=(PartitionSpec("core"),) * n_outs,
            check_rep=False,
        ),
        donate_argnums=donate,
        keep_unused=True,
    )

    def run(in_maps, timeit=False):
        concat_in = [
            np.concatenate([np.asarray(m[name]) for m in in_maps], axis=0)
            for name in in_names
        ]
        concat_in = [jax.device_put(a) for a in concat_in]
        zeros = [
            jax.device_put(
                np.zeros((NCORES * av.shape[0], *av.shape[1:]), av.dtype)
            )
            for av in out_avals
        ]
        for a in concat_in + zeros:
            a.block_until_ready()
        t0 = time.monotonic()
        out_arrs = sharded(*concat_in, *zeros)
        for o in out_arrs:
            o.block_until_ready()
        dt = time.monotonic() - t0
        res = [
            {
                name: np.asarray(out_arrs[i]).reshape(
                    NCORES, *out_avals[i].shape
                )[c]
                for i, name in enumerate(out_names)
            }
            for c in range(NCORES)
        ]
        if timeit:
            return res, dt
        return res

    _CACHE["runner"] = run
    _CACHE["nc"] = nc
    return run


def make_in_maps(inputs):
    x = np.asarray(inputs["x"], np.float32)
    shared = host_inputs(inputs)
    return [
        {"x": np.ascontiguousarray(x[c * BPC : (c + 1) * BPC]), **shared}
        for c in range(NCORES)
    ]


def kernel(**inputs):
    run = get_runner()
    in_maps = make_in_maps(inputs)
    res = run(in_maps)
    y = np.concatenate([np.asarray(r["y"]) for r in res], axis=0)
    return y.astype(np.float32)


# revision 16
# speedup vs baseline: 11530.8684x; 198.8084x over previous
"""Fused ViT-style transformer block on 8 TRN2 NeuronCores.

Sharding: data-parallel over batch (32 batches -> 4 per core). Each core runs
the full block (LN1 -> QKV -> attention -> proj -> residual -> LN2 -> MLP ->
residual) on its 4 batches. No collectives.

Layout strategy per batch (N=577 tokens, D=768, H=12 heads, Dh=64):
  - x, x2, y: natural [token, feature] fp32, resident in one SBUF tile
    (residuals applied in place).
  - LayerNorm affine (g, b) folded into the following matmul weights on the
    host, so on-chip LN is just (x - mu) * rstd.
  - h (LN out) transposed via PE to hT [feature, token]; qT/kT computed
    feature-major directly (lhsT = W chunk), v computed token-major with an
    appended ones column so the PV matmul also yields softmax row sums.
  - Scores S^T [key j, query i] = kT.T @ qT; softmax skips max-subtraction
    (scores ~ N(0,1) after 1/8 scaling, safe in fp32 exp); exp on ACT engine
    with the 1/8 scale folded in, output bf16.
  - o = P @ v accumulated over key chunks; normalized per-token by the
    reciprocal of the appended row-sum column (per-partition scalar).
  - Matmul operands bf16 (PSUM accumulation fp32), activations/stats fp32.
"""

import time

import numpy as np
import ml_dtypes
from contextlib import ExitStack

import concourse.bass as bass
import concourse.bacc as bacc
import concourse.tile as tile
from concourse import mybir
from concourse.masks import make_identity

F32 = mybir.dt.float32
BF16 = mybir.dt.bfloat16
AF = mybir.ActivationFunctionType
OP = mybir.AluOpType

B, N, D, H = 32, 577, 768, 12
DH = D // H            # 64
HID = 4 * D            # 3072
NCORES = 8
BPC = B // NCORES      # batches per core
P = 128
KD = D // P            # 6
KH = HID // P          # 24
NT = 5                 # token chunks per batch: 4x128 + 65
TSZ = [128, 128, 128, 128, 65]
TOF = [0, 128, 256, 384, 512]
EPS = 1e-5
SCALE = DH ** -0.5


def _bcast(ap_1d, p=P):
    """AP that reads a 1-D dram tensor broadcast across p partitions."""
    return bass.AP(
        tensor=ap_1d.tensor, offset=ap_1d.offset, ap=[[0, p]] + list(ap_1d.ap)
    )


def _layernorm(nc, stat, x_sb, out_sb, eps_sb):
    """out = (x - mean) * rsqrt(var + eps), per token chunk; out bf16."""
    for t in range(NT):
        ts_ = TSZ[t]
        st = stat.tile([P, 3, 6], F32, tag="bnst")
        mv = stat.tile([P, 2], F32, tag="bnmv")
        xin = x_sb[:ts_, t, :].rearrange("p (s q) -> p s q", s=3)
        for s in range(3):
            nc.vector.bn_stats(out=st[:ts_, s, :], in_=xin[:, s, :])
        nc.vector.bn_aggr(out=mv[:ts_], in_=st[:ts_])
        rstd = stat.tile([P, 1], F32, tag="rstd")
        nc.scalar.activation(
            out=rstd[:ts_], in_=mv[:ts_, 1:2], func=AF.Sqrt, bias=eps_sb[:ts_], scale=1.0
        )
        nc.vector.reciprocal(out=rstd[:ts_], in_=rstd[:ts_])
        nc.vector.tensor_scalar(
            out=out_sb[:ts_, t, :],
            in0=x_sb[:ts_, t, :],
            scalar1=mv[:ts_, 0:1],
            scalar2=rstd[:ts_],
            op0=OP.subtract,
            op1=OP.mult,
        )


def _transpose5x6(nc, ps_sm, ident, src_sb, dst_t6):
    """[token, 768] (5 chunks) -> [768, token] (6 chunks) via PE transposes."""
    for t in range(NT):
        ts_ = TSZ[t]
        for f in range(KD):
            tp = ps_sm.tile([P, P], BF16, tag="ps_sm")
            nc.tensor.transpose(
                out=tp[:, :ts_],
                in_=src_sb[:ts_, t, f * P : (f + 1) * P],
                identity=ident[:ts_, :ts_],
            )
            nc.vector.tensor_copy(
                out=dst_t6[:, f, TOF[t] : TOF[t] + ts_], in_=tp[:, :ts_]
            )


def _body(ctx, tc, d):
    nc = tc.nc

    const = ctx.enter_context(tc.tile_pool(name="const", bufs=1))
    xp = ctx.enter_context(tc.tile_pool(name="xp", bufs=2))
    hbf = ctx.enter_context(tc.tile_pool(name="hbf", bufs=2))
    t6 = ctx.enter_context(tc.tile_pool(name="t6", bufs=2))
    vvp = ctx.enter_context(tc.tile_pool(name="vv", bufs=1))
    qkp = ctx.enter_context(tc.tile_pool(name="qk", bufs=4))
    esp = ctx.enter_context(tc.tile_pool(name="es", bufs=1))
    h1p = ctx.enter_context(tc.tile_pool(name="h1", bufs=1))
    w1p = ctx.enter_context(tc.tile_pool(name="w1", bufs=4))
    stat = ctx.enter_context(tc.tile_pool(name="stat", bufs=4))
    ps_mm = ctx.enter_context(tc.tile_pool(name="ps_mm", bufs=2, space="PSUM"))
    ps_sm = ctx.enter_context(tc.tile_pool(name="ps_sm", bufs=4, space="PSUM"))

    # ---- one-time constants ----
    ident = const.tile([P, P], BF16)
    make_identity(nc, ident)
    eps_sb = const.tile([P, 1], F32)
    nc.vector.memset(eps_sb, EPS)

    wqkv_sb = const.tile([P, KD, 3 * D], BF16)
    nc.sync.dma_start(
        out=wqkv_sb, in_=d["w_qkv"][:].rearrange("(c p) f -> p c f", p=P)
    )
    wproj_sb = const.tile([P, KD, D], BF16)
    nc.sync.dma_start(
        out=wproj_sb, in_=d["w_proj"][:].rearrange("(c p) f -> p c f", p=P)
    )
    wfc2_sb = const.tile([P, KH, D], BF16)
    nc.sync.dma_start(
        out=wfc2_sb, in_=d["w_fc2"][:].rearrange("(c p) f -> p c f", p=P)
    )
    bqk_sb = const.tile([P, 12], F32)
    nc.sync.dma_start(out=bqk_sb, in_=d["b_qk"][:].rearrange("(c p) -> p c", p=P))
    bfc1_sb = const.tile([P, KH], F32)
    nc.sync.dma_start(out=bfc1_sb, in_=d["b_fc1"][:].rearrange("(c p) -> p c", p=P))
    bv_sb = const.tile([P, D], BF16)
    nc.sync.dma_start(out=bv_sb, in_=_bcast(d["b_v"][:]))
    bproj_sb = const.tile([P, D], BF16)
    nc.sync.dma_start(out=bproj_sb, in_=_bcast(d["b_proj"][:]))
    bfc2_sb = const.tile([P, D], BF16)
    nc.sync.dma_start(out=bfc2_sb, in_=_bcast(d["b_fc2"][:]))

    x_d, y_d = d["x"], d["y"]

    for b in range(BPC * d.get("_reps", 1)):
        b = b % BPC
        # ---- load x ----
        x_sb = xp.tile([P, NT, D], F32, tag="x")
        nc.sync.dma_start(
            out=x_sb[:, 0:4, :],
            in_=x_d[b, 0:512, :].rearrange("(c p) f -> p c f", p=P),
        )
        nc.sync.dma_start(out=x_sb[0:65, 4, :], in_=x_d[b, 512:577, :])

        # ---- LN1 -> h ----
        h_sb = hbf.tile([P, NT, D], BF16, tag="hbf")
        _layernorm(nc, stat, x_sb, h_sb, eps_sb)

        # ---- transpose h -> hT ----
        hT = t6.tile([P, KD, N], BF16, tag="t6")
        _transpose5x6(nc, ps_sm, ident, h_sb, hT)

        # ---- v = h @ Wv + bv (token-major), append ones column ----
        v_sb = vvp.tile([P, NT, H, DH + 1], BF16, tag="vv")
        for t in range(NT):
            ts_ = TSZ[t]
            pv = ps_mm.tile([P, D], F32, tag="ps_mm")
            for k in range(KD):
                for n0, n1 in ((0, 512), (512, 768)):
                    nc.tensor.matmul(
                        pv[:ts_, n0:n1],
                        lhsT=hT[:, k, TOF[t] : TOF[t] + ts_],
                        rhs=wqkv_sb[:, k, 2 * D + n0 : 2 * D + n1],
                        start=(k == 0),
                        stop=(k == KD - 1),
                    )
            for hh in range(H):
                nc.vector.tensor_add(
                    out=v_sb[:ts_, t, hh, 0:DH],
                    in0=pv[:ts_, hh * DH : (hh + 1) * DH],
                    in1=bv_sb[:ts_, hh * DH : (hh + 1) * DH],
                )
            nc.vector.memset(v_sb[:ts_, t, :, DH : DH + 1], 1.0)

        # ---- attention, two heads per 128-feature chunk ----
        o_sb = hbf.tile([P, NT, D], BF16, tag="hbf")
        for hp in range(KD):
            qT = qkp.tile([P, N], BF16, tag="qk")
            kT = qkp.tile([P, N], BF16, tag="qk")
            for dst, base, col in ((qT, hp * P, hp), (kT, D + hp * P, KD + hp)):
                pq = ps_mm.tile([P, N], F32, tag="ps_mm")
                for k in range(KD):
                    for n0, n1 in ((0, 512), (512, N)):
                        nc.tensor.matmul(
                            pq[:, n0:n1],
                            lhsT=wqkv_sb[:, k, base : base + P],
                            rhs=hT[:, k, n0:n1],
                            start=(k == 0),
                            stop=(k == KD - 1),
                        )
                nc.vector.tensor_scalar_add(
                    out=dst, in0=pq, scalar1=bqk_sb[:, col : col + 1]
                )
            for sub in range(2):
                hh = hp * 2 + sub
                head_q = qT[sub * DH : (sub + 1) * DH, :]
                head_k = kT[sub * DH : (sub + 1) * DH, :]
                es = esp.tile([P, NT, N], BF16, tag="es")
                for j in range(NT):
                    js = TSZ[j]
                    ps_s = ps_mm.tile([P, N], F32, tag="ps_mm")
                    for n0, n1 in ((0, 512), (512, N)):
                        nc.tensor.matmul(
                            ps_s[:js, n0:n1],
                            lhsT=head_k[:, TOF[j] : TOF[j] + js],
                            rhs=head_q[:, n0:n1],
                            start=True,
                            stop=True,
                        )
                    nc.scalar.activation(
                        out=es[:js, j, :], in_=ps_s[:js, :], func=AF.Exp, scale=SCALE
                    )
                for t in range(NT):
                    ts_ = TSZ[t]
                    po = ps_sm.tile([P, DH + 1], F32, tag="ps_sm")
                    for j in range(NT):
                        js = TSZ[j]
                        nc.tensor.matmul(
                            po[:ts_, :],
                            lhsT=es[:js, j, TOF[t] : TOF[t] + ts_],
                            rhs=v_sb[:js, j, hh, :],
                            start=(j == 0),
                            stop=(j == NT - 1),
                        )
                    rcp = stat.tile([P, 1], F32, tag="rcp")
                    nc.vector.reciprocal(out=rcp[:ts_], in_=po[:ts_, DH : DH + 1])
                    nc.vector.tensor_scalar_mul(
                        out=o_sb[:ts_, t, hh * DH : (hh + 1) * DH],
                        in0=po[:ts_, 0:DH],
                        scalar1=rcp[:ts_],
                    )

        # ---- transpose o -> oT ----
        oT = t6.tile([P, KD, N], BF16, tag="t6")
        _transpose5x6(nc, ps_sm, ident, o_sb, oT)

        # ---- proj + residual into x_sb (x2 = x + o @ Wp + bp) ----
        for t in range(NT):
            ts_ = TSZ[t]
            pp = ps_mm.tile([P, D], F32, tag="ps_mm")
            for k in range(KD):
                for n0, n1 in ((0, 512), (512, 768)):
                    nc.tensor.matmul(
                        pp[:ts_, n0:n1],
                        lhsT=oT[:, k, TOF[t] : TOF[t] + ts_],
                        rhs=wproj_sb[:, k, n0:n1],
                        start=(k == 0),
                        stop=(k == KD - 1),
                    )
            nc.vector.tensor_add(
                out=x_sb[:ts_, t, :], in0=x_sb[:ts_, t, :], in1=pp[:ts_, :]
            )
            nc.vector.tensor_add(
                out=x_sb[:ts_, t, :], in0=x_sb[:ts_, t, :], in1=bproj_sb[:ts_, :]
            )

        # ---- LN2 -> h2, transpose ----
        h2_sb = hbf.tile([P, NT, D], BF16, tag="hbf")
        _layernorm(nc, stat, x_sb, h2_sb, eps_sb)
        h2T = t6.tile([P, KD, N], BF16, tag="t6")
        _transpose5x6(nc, ps_sm, ident, h2_sb, h2T)

        # ---- fc1 (feature-major out) + relu6 -> h1T; wfc1 streamed ----
        h1T = h1p.tile([P, KH, N], BF16, tag="h1")
        for f in range(KH):
            w1t = w1p.tile([P, KD, P], BF16, tag="w1")
            nc.sync.dma_start(
                out=w1t,
                in_=d["w_fc1"][:, f * P : (f + 1) * P].rearrange(
                    "(c p) f -> p c f", p=P
                ),
            )
            pf = ps_mm.tile([P, N], F32, tag="ps_mm")
            for k in range(KD):
                for n0, n1 in ((0, 512), (512, N)):
                    nc.tensor.matmul(
                        pf[:, n0:n1],
                        lhsT=w1t[:, k, :],
                        rhs=h2T[:, k, n0:n1],
                        start=(k == 0),
                        stop=(k == KD - 1),
                    )
            nc.scalar.activation(
                out=pf, in_=pf, func=AF.Relu, bias=bfc1_sb[:, f : f + 1], scale=1.0
            )
            nc.vector.tensor_scalar_min(out=h1T[:, f, :], in0=pf, scalar1=6.0)

        # ---- fc2 + residual -> y (into x_sb), DMA out ----
        for t in range(NT):
            ts_ = TSZ[t]
            pf2 = ps_mm.tile([P, D], F32, tag="ps_mm")
            for k in range(KH):
                for n0, n1 in ((0, 512), (512, 768)):
                    nc.tensor.matmul(
                        pf2[:ts_, n0:n1],
                        lhsT=h1T[:, k, TOF[t] : TOF[t] + ts_],
                        rhs=wfc2_sb[:, k, n0:n1],
                        start=(k == 0),
                        stop=(k == KH - 1),
                    )
            nc.vector.tensor_add(
                out=x_sb[:ts_, t, :], in0=x_sb[:ts_, t, :], in1=pf2[:ts_, :]
            )
            nc.vector.tensor_add(
                out=x_sb[:ts_, t, :], in0=x_sb[:ts_, t, :], in1=bfc2_sb[:ts_, :]
            )
        nc.sync.dma_start(
            out=y_d[b, 0:512, :].rearrange("(c p) f -> p c f", p=P),
            in_=x_sb[:, 0:4, :],
        )
        nc.sync.dma_start(out=y_d[b, 512:577, :], in_=x_sb[0:65, 4, :])


def build_nc(reps=1):
    nc = bacc.Bacc("TRN2", target_bir_lowering=False, debug=False)
    d = {
        "_reps": reps,
        "x": nc.dram_tensor("x", [BPC, N, D], F32, kind="ExternalInput"),
        "w_qkv": nc.dram_tensor("w_qkv", [D, 3 * D], BF16, kind="ExternalInput"),
        "b_qk": nc.dram_tensor("b_qk", [2 * D], F32, kind="ExternalInput"),
        "b_v": nc.dram_tensor("b_v", [D], BF16, kind="ExternalInput"),
        "w_proj": nc.dram_tensor("w_proj", [D, D], BF16, kind="ExternalInput"),
        "b_proj": nc.dram_tensor("b_proj", [D], BF16, kind="ExternalInput"),
        "w_fc1": nc.dram_tensor("w_fc1", [D, HID], BF16, kind="ExternalInput"),
        "b_fc1": nc.dram_tensor("b_fc1", [HID], F32, kind="ExternalInput"),
        "w_fc2": nc.dram_tensor("w_fc2", [HID, D], BF16, kind="ExternalInput"),
        "b_fc2": nc.dram_tensor("b_fc2", [D], BF16, kind="ExternalInput"),
        "y": nc.dram_tensor("y", [BPC, N, D], F32, kind="ExternalOutput"),
    }
    with tile.TileContext(nc) as tc:
        with ExitStack() as ctx:
            _body(ctx, tc, d)
    nc.compile()
    return nc


def host_inputs(inputs):
    """Fold LN affine params into weights; cast matmul operands to bf16."""
    bf = ml_dtypes.bfloat16
    f32 = np.float32
    g1 = np.asarray(inputs["ln1_g"], f32)
    b1 = np.asarray(inputs["ln1_b"], f32)
    g2 = np.asarray(inputs["ln2_g"], f32)
    b2 = np.asarray(inputs["ln2_b"], f32)
    w_qkv = np.asarray(inputs["w_qkv"], f32)
    w_fc1 = np.asarray(inputs["w_fc1"], f32)
    b_qkv_eff = np.asarray(inputs["b_qkv"], f32) + b1 @ w_qkv
    b_fc1_eff = np.asarray(inputs["b_fc1"], f32) + b2 @ w_fc1
    return {
        "w_qkv": (g1[:, None] * w_qkv).astype(bf),
        "b_qk": b_qkv_eff[: 2 * D].astype(f32),
        "b_v": b_qkv_eff[2 * D :].astype(bf),
        "w_proj": np.asarray(inputs["w_proj"], f32).astype(bf),
        "b_proj": np.asarray(inputs["b_proj"], f32).astype(bf),
        "w_fc1": (g2[:, None] * w_fc1).astype(bf),
        "b_fc1": b_fc1_eff.astype(f32),
        "w_fc2": np.asarray(inputs["w_fc2"], f32).astype(bf),
        "b_fc2": np.asarray(inputs["b_fc2"], f32).astype(bf),
    }


_CACHE = {}


def get_runner(reps=1):
    """Build (once) the bass module and a persistent 8-core PJRT runner.

    Mirrors bass2jax.run_bass_via_pjrt's multi-core branch, but caches the
    jitted shard_map callable so repeat calls don't re-trace/re-compile.
    """
    key = ("runner", reps)
    if key in _CACHE:
        return _CACHE[key]

    import jax
    from jax.sharding import Mesh, PartitionSpec
    from jax.experimental.shard_map import shard_map
    from concourse import bass2jax, mybir as mb

    bass2jax.install_neuronx_cc_hook()
    nc = build_nc(reps=reps)

    partition_name = nc.partition_id_tensor.name if nc.partition_id_tensor else None
    in_names, out_names, out_avals = [], [], []
    for alloc in nc.m.functions[0].allocations:
        if not isinstance(alloc, mb.MemoryLocationSet):
            continue
        name = alloc.memorylocations[0].name
        if alloc.kind == "ExternalInput":
            if name != partition_name:
                in_names.append(name)
        elif alloc.kind == "ExternalOutput":
            out_names.append(name)
            out_avals.append(
                jax.core.ShapedArray(tuple(alloc.tensor_shape), mb.dt.np(alloc.dtype))
            )
    n_params = len(in_names)
    n_outs = len(out_names)
    all_names = in_names + out_names
    if partition_name is not None:
        all_names = all_names + [partition_name]

    def _body(*args):
        operands = list(args)
        if partition_name is not None:
            operands.append(bass2jax.partition_id_tensor())
        return tuple(
            bass2jax._bass_exec_p.bind(
                *operands,
                out_avals=tuple(out_avals),
                in_names=tuple(all_names),
                out_names=tuple(out_names),
                lowering_input_output_aliases=(),
                sim_require_finite=True,
                sim_require_nnan=True,
                nc=nc,
            )
        )

    devices = jax.devices()[:NCORES]
    mesh = Mesh(np.asarray(devices), ("core",))
    donate = tuple(range(n_params, n_params + n_outs))
    sharded = jax.jit(
        shard_map(
            _body,
            mesh=mesh,
            in_specs=(PartitionSpec("core"),) * (n_params + n_outs),
            out_specs=(PartitionSpec("core"),) * n_outs,
            check_rep=False,
        ),
        donate_argnums=donate,
        keep_unused=True,
    )

    def run(in_maps, timeit=False):
        concat_in = [
            np.concatenate([np.asarray(m[name]) for m in in_maps], axis=0)
            for name in in_names
        ]
        concat_in = [jax.device_put(a) for a in concat_in]
        zeros = [
            jax.device_put(
                np.zeros((NCORES * av.shape[0], *av.shape[1:]), av.dtype)
            )
            for av in out_avals
        ]
        for a in concat_in + zeros:
            a.block_until_ready()
        t0 = time.monotonic()
        out_arrs = sharded(*concat_in, *zeros)
        for o in out_arrs:
            o.block_until_ready()
        dt = time.monotonic() - t0
        res = [
            {
                name: np.asarray(out_arrs[i]).reshape(
                    NCORES, *out_avals[i].shape
                )[c]
                for i, name in enumerate(out_names)
            }
            for c in range(NCORES)
        ]
        if timeit:
            return res, dt
        return res

    _CACHE[key] = run
    return run


def make_in_maps(inputs):
    x = np.asarray(inputs["x"], np.float32)
    shared = host_inputs(inputs)
    return [
        {"x": np.ascontiguousarray(x[c * BPC : (c + 1) * BPC]), **shared}
        for c in range(NCORES)
    ]


def kernel(**inputs):
    run = get_runner()
    in_maps = make_in_maps(inputs)
    res = run(in_maps)
    y = np.concatenate([np.asarray(r["y"]) for r in res], axis=0)
    return y.astype(np.float32)
